# revision 3
# baseline (speedup 1.0000x reference)
"""GQA attention kernel for Trainium2, 8 NeuronCores.

Problem: x[2,2048,2048] @ Wq/Wk/Wv -> grouped-query attention (16 q heads,
4 kv groups, head_dim 128, causal) -> @ Wo + bo.

Sharding: (batch b in 0..1) x (kv group g in 0..3) -> 8 cores.
Each core computes the full attention for its (b, g): 4 query heads sharing
one kv head, then a row-parallel partial of the output projection
(ctx_g @ Wo[g*512:(g+1)*512, :]). Host sums the 4 group partials per batch
and adds the bias.

Perf design (282us baseline for this file's ancestor -> this version):
  - softmax denominator accumulated on DVE (acc += ex per key tile) instead
    of a PE matmul per (r, jk); only 4 sel_ones matmuls per block on the
    final accumulator (-26us of PE streaming).
  - exp merged across head pairs: scores for (r, r+1) land in one 2-bank
    PSUM tile, one ACT instruction exponentiates [128, 2, w] (fixed ~260ns
    ACT overhead amortized; ACT was becoming the attention pacer).
  - projection matmuls for block ib+1 are interleaved at ~4-MM granularity
    into attention(ib)'s key loop, so the in-order PE queue always has
    independent work while ACT chews on exp.
  - warmup matmuls on a zeroed tile run during the initial DMA wait: HAM
    un-throttles (1.2->2.4GHz) before the first real matmul.
  - startup DMA descriptors split across the Sync and Scalar DGE queues
    (descriptor issue is ~650ns each, serialized per engine).
  - output written bf16 (host sums partials in f32): halves out DMA.
  - all matmul inputs bf16; every stationary operand 128x128.
"""

import os

import ml_dtypes
import numpy as np

import concourse.bass as bass
from concourse import bacc
import concourse.bass_isa as bass_isa
import concourse.mybir as mybir
from concourse.bass_utils import run_bass_kernel_spmd
from concourse.tile import TileContext

B, N, D = 2, 2048, 2048
G, REP, HD = 4, 4, 128
E = REP * HD  # 512 q-dims per group
P = 128
IB = 512  # i-block (query block) size
NBLK = N // IB  # 4
NCT = D // P  # 16 contraction tiles
NJT = N // P  # 16 key tiles
SCALE = 1.0 / float(np.sqrt(HD))

F32 = mybir.dt.float32
BF16 = mybir.dt.bfloat16

NWARM_512 = 6
NWARM_128 = 12

_LAST_RESULT = None  # test.py reads exec_time_ns from here


def build_bass():
    nc = bacc.Bacc()
    # All inputs bf16, laid out [partition, chunk, free] on the host so each
    # tensor needs few DMA descriptors.
    xT = nc.dram_tensor("xT", [P, NCT, N], BF16, kind="ExternalInput")
    wq = nc.dram_tensor("wq", [P, NCT, E], BF16, kind="ExternalInput")
    wk = nc.dram_tensor("wk", [P, NCT, HD], BF16, kind="ExternalInput")
    wv = nc.dram_tensor("wv", [P, NCT, HD], BF16, kind="ExternalInput")
    wo = nc.dram_tensor("wo", [P, REP, D], BF16, kind="ExternalInput")
    out = nc.dram_tensor("out", [N, D], BF16, kind="ExternalOutput")

    with TileContext(nc) as tc:
        build_tile_kernel(nc, tc, xT, wq, wk, wv, wo, out)
    nc.finalize()
    return nc


def build_tile_kernel(nc, tc, xT, wq, wk, wv, wo, out):
    import contextlib

    ctx = contextlib.ExitStack()
    with ctx:
        persist = ctx.enter_context(tc.tile_pool(name="persist", bufs=1))
        weights = ctx.enter_context(tc.tile_pool(name="weights", bufs=1))
        work = ctx.enter_context(tc.tile_pool(name="work", bufs=2))
        # PSUM budget (8 banks): scores 2 + ctx 4 + proj/outproj 2
        psum_sb = ctx.enter_context(
            tc.tile_pool(name="psum_sb", bufs=1, space="PSUM")
        )
        psum_cb = ctx.enter_context(
            tc.tile_pool(name="psum_cb", bufs=1, space="PSUM")
        )
        psum_pp = ctx.enter_context(
            tc.tile_pool(name="psum_pp", bufs=2, space="PSUM")
        )

        # ---- warmup: matmuls on zeroed data while input DMAs fly ----
        warm = persist.tile([P, IB], BF16, name="warm")
        nc.vector.memset(warm, 0.0)
        for i in range(NWARM_512):
            psw = psum_pp.tile([P, IB], F32, name=f"warmA{i}", tag="pp")
            nc.tensor.matmul(
                psw, lhsT=warm[:, 0:P], rhs=warm, start=True, stop=True,
                skip_group_check=True,
            )
        for i in range(NWARM_128):
            psw = psum_pp.tile([P, IB], F32, name=f"warmB{i}", tag="pp")
            nc.tensor.matmul(
                psw[:, 0:P], lhsT=warm[:, 0:P], rhs=warm[:, 0:P],
                start=True, stop=True, skip_group_check=True,
            )

        # ---- startup DMAs: alternate Sync / Scalar descriptor queues ----
        wk_all = weights.tile([P, NCT, HD], BF16, name="wk_all")
        xts = {}
        xts[0] = work.tile([P, NCT, IB], BF16, name="xt0", tag="xt", bufs=2)
        nc.sync.dma_start(out=wk_all[:, 0:8, :], in_=wk[:, 0:8, :])
        nc.scalar.dma_start(out=xts[0][:, 0:2, :], in_=xT[:, 0:2, 0:IB])
        nc.sync.dma_start(out=xts[0][:, 2:4, :], in_=xT[:, 2:4, 0:IB])
        nc.scalar.dma_start(out=xts[0][:, 4:6, :], in_=xT[:, 4:6, 0:IB])
        nc.sync.dma_start(out=xts[0][:, 6:8, :], in_=xT[:, 6:8, 0:IB])
        nc.scalar.dma_start(out=xts[0][:, 8:12, :], in_=xT[:, 8:12, 0:IB])
        nc.sync.dma_start(out=xts[0][:, 12:16, :], in_=xT[:, 12:16, 0:IB])
        nc.scalar.dma_start(out=wk_all[:, 8:16, :], in_=wk[:, 8:16, :])
        wv_all = weights.tile([P, NCT, HD], BF16, name="wv_all")
        nc.sync.dma_start(out=wv_all[:, :, :], in_=wv[:, :, :])
        wq_all = weights.tile([P, NCT, E], BF16, name="wq_all")
        nc.scalar.dma_start(out=wq_all[:, 0:8, :], in_=wq[:, 0:8, :])
        nc.sync.dma_start(out=wq_all[:, 8:16, :], in_=wq[:, 8:16, :])
        wo_all = weights.tile([P, REP, D], BF16, name="wo_all")
        nc.scalar.dma_start(out=wo_all[:, 0:2, :], in_=wo[:, 0:2, :])
        nc.sync.dma_start(out=wo_all[:, 2:4, :], in_=wo[:, 2:4, :])

        def load_xt(k):
            t = work.tile([P, NCT, IB], BF16, name=f"xt{k}", tag="xt", bufs=2)
            isl = slice(k * IB, (k + 1) * IB)
            nc.sync.dma_start(out=t[:, 0:8, :], in_=xT[:, 0:8, isl])
            nc.sync.dma_start(out=t[:, 8:16, :], in_=xT[:, 8:16, isl])
            xts[k] = t

        load_xt(1)

        # ---- constants ----
        # sel_ones[r]: [128,128] bf16, column r all ones (den matmul lhsT).
        sel_ones = []
        for r in range(REP):
            t = persist.tile([P, P], BF16, name=f"selo{r}", tag="selo", bufs=REP)
            nc.vector.memset(t, 0.0)
            nc.vector.memset(t[:, r : r + 1], 1.0)
            sel_ones.append(t)
        # sel4[r]: [128,128] bf16, row r all ones (reciprocal broadcast lhsT).
        sel4 = []
        for r in range(REP):
            t = persist.tile([P, P], BF16, name=f"sel4{r}", tag="sel4", bufs=REP)
            nc.vector.memset(t, 1.0)
            nc.gpsimd.affine_select(
                out=t,
                in_=t,
                compare_op=mybir.AluOpType.is_equal,
                fill=0.0,
                base=-r,
                pattern=[[0, P]],
                channel_multiplier=1,
            )
            sel4.append(t)

        rec4b = persist.tile([P, IB], BF16, name="rec4b")  # rows 0:4 live
        nc.vector.memset(rec4b, 0.0)
        kT = persist.tile([P, N], BF16)  # [d, i]
        v_sb = [
            persist.tile([P, HD], BF16, name=f"v{jt}", tag="v", bufs=NJT)
            for jt in range(NJT)
        ]

        def proj_items(ib, xt_all, qts_out):
            """K/V/Q projections for block ib; yields every few matmuls so
            the caller can interleave them into the attention loop."""
            isl = slice(ib * IB, (ib + 1) * IB)
            psk = psum_pp.tile([P, IB], F32, name=f"psk{ib}", tag="pp")
            for ct in range(NCT):
                nc.tensor.matmul(
                    psk, lhsT=wk_all[:, ct, :], rhs=xt_all[:, ct, :],
                    start=(ct == 0), stop=(ct == NCT - 1),
                    skip_group_check=True,
                )
                if ct % 4 == 3:
                    yield
            nc.scalar.copy(kT[:, isl], psk)
            yield
            # V directly in natural [j, d] layout: lhsT = a 128-query strip
            # of xT (contraction on partitions), rhs = Wv tile. No transpose.
            for sub in range(IB // P):
                jt = ib * (IB // P) + sub
                psv = psum_pp.tile([P, IB], F32, name=f"psv{jt}", tag="pp")
                for ct in range(NCT):
                    nc.tensor.matmul(
                        psv[:, 0:HD],
                        lhsT=xt_all[:, ct, sub * P : (sub + 1) * P],
                        rhs=wv_all[:, ct, :],
                        start=(ct == 0), stop=(ct == NCT - 1),
                        skip_group_check=True,
                    )
                    if ct % 8 == 7:
                        yield
                nc.vector.tensor_copy(v_sb[jt], psv[:, 0:HD])
                yield
            for r in range(REP):
                psq = psum_pp.tile([P, IB], F32, name=f"psq{ib}_{r}", tag="pp")
                for ct in range(NCT):
                    nc.tensor.matmul(
                        psq,
                        lhsT=wq_all[:, ct, r * P : (r + 1) * P],
                        rhs=xt_all[:, ct, :],
                        start=(ct == 0), stop=(ct == NCT - 1),
                        skip_group_check=True,
                    )
                    if ct % 4 == 3:
                        yield
                qt = work.tile([P, IB], BF16, name=f"qT{ib}_{r}", tag="qT", bufs=8)
                if r % 2 == 0:
                    nc.scalar.copy(qt, psq)
                else:
                    nc.vector.tensor_copy(qt, psq)
                qts_out.append(qt)
                yield

        def drain(gen):
            for _ in gen:
                pass

        # ---- prologue: block 0 projections, not interleaved ----
        qts_cur = []
        drain(proj_items(0, xts[0], qts_cur))

        for ib in range(NBLK):
            if ib + 2 < NBLK:
                load_xt(ib + 2)
            qts_next = []
            if ib + 1 < NBLK:
                gen = proj_items(ib + 1, xts[ib + 1], qts_next)
            else:
                gen = iter(())

            def fill(n):
                for _ in range(n):
                    next(gen, None)

            # ============ attention for this query block ============
            njt = (ib + 1) * (IB // P)  # causal: key tiles 0..njt-1
            # last diag group(s) feed the denominator via direct PE matmuls
            # (skips the DVE accumulator on the block's critical tail)
            direct_from = 2 if ib == NBLK - 1 else 3
            ps_ctx = psum_cb.tile([P, REP, IB], F32, name=f"ctx{ib}", tag="cb")
            acc = work.tile([P, REP, IB], F32, name=f"acc{ib}", tag="acc", bufs=1)
            den = None
            direct_ex = []
            for jk in range(njt):
                m = jk - (njt - 4)  # >= 0 on the diagonal strip
                i0 = max(m, 0) * P  # live columns: i >= 128*m
                ex = work.tile(
                    [P, REP, IB], BF16, name=f"ex{ib}_{jk}", tag="ex", bufs=3
                )
                for h in (0, 1):
                    sb = psum_sb.tile(
                        [P, 2, IB], F32, name=f"sb{ib}_{jk}_{h}", tag="sb"
                    )
                    for rr in (0, 1):
                        r = 2 * h + rr
                        nc.tensor.matmul(
                            sb[:, rr, i0:],
                            lhsT=kT[:, jk * P : (jk + 1) * P],
                            rhs=qts_cur[r][:, i0:],
                            start=True, stop=True, skip_group_check=True,
                        )
                    nc.scalar.activation(
                        ex[:, 2 * h : 2 * h + 2, i0:],
                        sb[:, :, i0:],
                        mybir.ActivationFunctionType.Exp,
                        scale=SCALE,
                    )
                    if m >= 0:
                        for rr in (0, 1):
                            r = 2 * h + rr
                            # triangular strip: keep where (i - i0) - j >= 0
                            nc.gpsimd.affine_select(
                                out=ex[:, r, i0 : i0 + P],
                                in_=ex[:, r, i0 : i0 + P],
                                compare_op=mybir.AluOpType.is_ge,
                                fill=0.0,
                                base=0,
                                pattern=[[1, P]],
                                channel_multiplier=-1,
                            )
                    for rr in (0, 1):
                        r = 2 * h + rr
                        nc.tensor.matmul(
                            ps_ctx[:, r, i0:],
                            lhsT=v_sb[jk],
                            rhs=ex[:, r, i0:],
                            start=(jk == 0), stop=(jk == njt - 1),
                            skip_group_check=True,
                        )
                    fill(1)
                if m < direct_from:
                    if jk == 0:
                        nc.vector.tensor_copy(acc, ex)
                    else:
                        nc.vector.tensor_add(
                            acc[:, :, i0:], acc[:, :, i0:], ex[:, :, i0:]
                        )
                else:
                    direct_ex.append((ex, i0))
                if m == direct_from - 1:
                    # denominator from the accumulator; diag tail comes from
                    # direct matmuls on the remaining ex tiles below.
                    acc_b = work.tile(
                        [P, REP, IB], BF16, name=f"accb{ib}", tag="accb", bufs=1
                    )
                    nc.vector.tensor_copy(acc_b, acc)
                    den = psum_pp.tile([P, IB], F32, name=f"den{ib}", tag="pp")
                    for r in range(REP):
                        nc.tensor.matmul(
                            den,
                            lhsT=sel_ones[r],
                            rhs=acc_b[:, r, :],
                            start=(r == 0), stop=False,
                            skip_group_check=True,
                        )
                if m >= direct_from:
                    last = m == 3
                    ex_t, exi0 = direct_ex[-1]
                    for r in range(REP):
                        nc.tensor.matmul(
                            den[:, exi0:],
                            lhsT=sel_ones[r],
                            rhs=ex_t[:, r, exi0:],
                            start=False, stop=(last and r == REP - 1),
                            skip_group_check=True,
                        )

            # ============ normalize: recip + broadcast ============
            rec4 = work.tile([REP, IB], F32, name=f"rec4{ib}", tag="rec4", bufs=2)
            nc.vector.reciprocal_approx_fast(out=rec4, in_=den[0:REP, :])
            nc.vector.tensor_copy(rec4b[0:REP, :], rec4)
            fill(2)
            cns = []
            for h in (0, 1):
                rbp = psum_sb.tile([P, 2, IB], F32, name=f"rb{ib}_{h}", tag="sb")
                for rr in (0, 1):
                    nc.tensor.matmul(
                        rbp[:, rr, :], lhsT=sel4[2 * h + rr], rhs=rec4b,
                        start=True, stop=True, skip_group_check=True,
                    )
                rbs = work.tile(
                    [P, 2, IB], BF16, name=f"rbs{ib}_{h}", tag="rbs", bufs=2
                )
                nc.scalar.copy(rbs, rbp)
                fill(2)
                for rr in (0, 1):
                    r = 2 * h + rr
                    cn = work.tile([P, IB], BF16, name=f"cn{ib}_{r}", tag="cn", bufs=8)
                    nc.vector.tensor_mul(cn, ps_ctx[:, r, :], rbs[:, rr, :])
                    cns.append(cn)
            drain(gen)

            # ============ output projection ============
            for sub in range(IB // P):
                it = ib * (IB // P) + sub
                ssl = slice(sub * P, (sub + 1) * P)
                for half in range(2):
                    o2 = work.tile(
                        [P, 2 * IB], BF16, name=f"o{it}_{half}", tag="osb", bufs=4
                    )
                    for k in range(2):
                        ot = 2 * half + k
                        pso = psum_pp.tile(
                            [P, IB], F32, name=f"pso{it}_{ot}", tag="pp"
                        )
                        for r in range(REP):
                            nc.tensor.matmul(
                                pso,
                                lhsT=cns[r][:, ssl],
                                rhs=wo_all[:, r, ot * IB : (ot + 1) * IB],
                                start=(r == 0), stop=(r == REP - 1),
                                skip_group_check=True,
                            )
                        if (it + ot) % 2 == 0:
                            nc.vector.tensor_copy(o2[:, k * IB : (k + 1) * IB], pso)
                        else:
                            nc.scalar.copy(o2[:, k * IB : (k + 1) * IB], pso)
                    nc.sync.dma_start(
                        out=out[
                            it * P : (it + 1) * P,
                            half * 2 * IB : (half + 1) * 2 * IB,
                        ],
                        in_=o2,
                    )
            qts_cur = qts_next


_NC_CACHE = None


def kernel(x, Wq, Wk, Wv, Wo, bo):
    global _LAST_RESULT, _NC_CACHE
    x = np.asarray(x, dtype=np.float32)
    Wq = np.asarray(Wq, dtype=np.float32)
    Wk = np.asarray(Wk, dtype=np.float32)
    Wv = np.asarray(Wv, dtype=np.float32)
    Wo = np.asarray(Wo, dtype=np.float32)
    bo = np.asarray(bo, dtype=np.float32)

    if _NC_CACHE is None:
        _NC_CACHE = build_bass()
    nc = _NC_CACHE

    def chunked(a, pdim):
        # [pdim*nchunk, F] -> [pdim, nchunk, F] bf16, partition-major
        nchunk = a.shape[0] // pdim
        return np.ascontiguousarray(
            a.reshape(nchunk, pdim, a.shape[1]).transpose(1, 0, 2)
        ).astype(ml_dtypes.bfloat16)

    in_maps = []
    for core in range(8):
        b, g = core // G, core % G
        in_maps.append(
            {
                "xT": chunked(np.ascontiguousarray(x[b].T), P),
                "wq": chunked(Wq[:, g * E : (g + 1) * E], P),
                "wk": chunked(Wk[:, g * HD : (g + 1) * HD], P),
                "wv": chunked(Wv[:, g * HD : (g + 1) * HD], P),
                "wo": chunked(Wo[g * E : (g + 1) * E, :], P),
            }
        )
    res = run_bass_kernel_spmd(
        nc,
        in_maps,
        list(range(8)),
        trace=bool(os.environ.get("BASS_TRACE")),
    )
    _LAST_RESULT = res
    partials = np.stack(
        [np.asarray(r["out"]).astype(np.float32) for r in res.results]
    )  # [8, N, D]
    full = partials.reshape(B, G, N, D).sum(axis=1) + bo[None, None, :]
    return full.astype(np.float32)


# revision 9
# speedup vs baseline: 1.1664x; 1.1664x over previous
"""GQA attention kernel for Trainium2, 8 NeuronCores.

Problem: x[2,2048,2048] @ Wq/Wk/Wv -> grouped-query attention (16 q heads,
4 kv groups, head_dim 128, causal) -> @ Wo + bo.

Sharding: (batch b in 0..1) x (kv group g in 0..3) -> 8 cores.
Each core computes the full attention for its (b, g): 4 query heads sharing
one kv head, then a row-parallel partial of the output projection
(ctx_g @ Wo[g*512:(g+1)*512, :]). Host sums the 4 group partials per batch
and adds the bias.

Perf design (282us baseline for this file's ancestor -> this version):
  - softmax denominator accumulated on DVE (acc += ex per key tile) instead
    of a PE matmul per (r, jk); only a few sel_ones matmuls per block on the
    final accumulator (-26us of PE streaming).
  - exp merged across head pairs: scores for (r, r+1) land in one 2-bank
    PSUM tile, one ACT instruction exponentiates [128, 2, w] (fixed ~260ns
    ACT overhead amortized; ACT paces the attention inner loop).
  - the in-order PE queue is never left waiting on ACT: independent matmuls
    (outproj of block ib-1, then projections of block ib+1) are emitted in
    ~4-MM chunks BETWEEN each exp and its dependent ctx matmuls. Outproj
    lagging one block is what gives the final block filler too - otherwise
    the PE goes sparse and the HAM clock-gate drops it to 1.2 GHz.
  - warmup matmuls on a zeroed tile run during the initial DMA wait so HAM
    un-throttles before the first real matmul.
  - startup-critical DMAs (wk, x block 0) get the DMA rings to themselves;
    wv/wq/wo/x1 are held back ~5-14us via tile_wait_until.
  - output written bf16 (host sums partials in f32): halves out DMA.
  - all matmul inputs bf16; every stationary operand 128x128.
"""

import os
from itertools import chain

import ml_dtypes
import numpy as np

import concourse.bass as bass
from concourse import bacc
import concourse.bass_isa as bass_isa
import concourse.mybir as mybir
from concourse.bass_utils import run_bass_kernel_spmd
from concourse.tile import TileContext

B, N, D = 2, 2048, 2048
G, REP, HD = 4, 4, 128
E = REP * HD  # 512 q-dims per group
P = 128
IB = 512  # i-block (query block) size
NBLK = N // IB  # 4
NCT = D // P  # 16 contraction tiles
NJT = N // P  # 16 key tiles
SCALE = 1.0 / float(np.sqrt(HD))

F32 = mybir.dt.float32
BF16 = mybir.dt.bfloat16

NWARM_512 = 6
NWARM_128 = 12

_LAST_RESULT = None  # test.py reads exec_time_ns from here


def build_bass():
    nc = bacc.Bacc()
    # All inputs bf16, laid out [partition, chunk, free] on the host so each
    # tensor needs few DMA descriptors.
    xT = nc.dram_tensor("xT", [P, NCT, N], BF16, kind="ExternalInput")
    wq = nc.dram_tensor("wq", [P, NCT, E], BF16, kind="ExternalInput")
    wk = nc.dram_tensor("wk", [P, NCT, HD], BF16, kind="ExternalInput")
    wv = nc.dram_tensor("wv", [P, NCT, HD], BF16, kind="ExternalInput")
    wo = nc.dram_tensor("wo", [P, REP, D], BF16, kind="ExternalInput")
    out = nc.dram_tensor("out", [N, D], BF16, kind="ExternalOutput")

    with TileContext(nc) as tc:
        build_tile_kernel(nc, tc, xT, wq, wk, wv, wo, out)
    nc.finalize()
    return nc


def roundrobin(*gens):
    gens = [g for g in gens if g is not None]
    while gens:
        nxt = []
        for g in gens:
            try:
                next(g)
            except StopIteration:
                continue
            nxt.append(g)
            yield
        gens = nxt


def build_tile_kernel(nc, tc, xT, wq, wk, wv, wo, out):
    import contextlib

    ctx = contextlib.ExitStack()
    with ctx:
        persist = ctx.enter_context(tc.tile_pool(name="persist", bufs=1))
        weights = ctx.enter_context(tc.tile_pool(name="weights", bufs=1))
        work = ctx.enter_context(tc.tile_pool(name="work", bufs=2))
        # PSUM budget (8 banks): scores 2 + ctx 4 + proj/outproj 2
        psum_sb = ctx.enter_context(
            tc.tile_pool(name="psum_sb", bufs=1, space="PSUM")
        )
        psum_cb = ctx.enter_context(
            tc.tile_pool(name="psum_cb", bufs=1, space="PSUM")
        )
        psum_pp = ctx.enter_context(
            tc.tile_pool(name="psum_pp", bufs=2, space="PSUM")
        )

        # ---- warmup: matmuls on zeroed data while input DMAs fly ----
        warm = persist.tile([P, IB], BF16, name="warm")
        nc.vector.memset(warm, 0.0)
        for i in range(NWARM_512):
            psw = psum_pp.tile([P, IB], F32, name=f"warmA{i}", tag="pp")
            nc.tensor.matmul(
                psw, lhsT=warm[:, 0:P], rhs=warm, start=True, stop=True,
                skip_group_check=True,
            )
        for i in range(NWARM_128):
            psw = psum_pp.tile([P, IB], F32, name=f"warmB{i}", tag="pp")
            nc.tensor.matmul(
                psw[:, 0:P], lhsT=warm[:, 0:P], rhs=warm[:, 0:P],
                start=True, stop=True, skip_group_check=True,
            )

        # ---- startup DMAs ----
        # Criticals first (wk + x block 0) with the rings to themselves;
        # the rest is staggered so it doesn't steal bandwidth from them.
        wk_all = weights.tile([P, NCT, HD], BF16, name="wk_all")
        xts = {}
        xts[0] = work.tile([P, NCT, IB], BF16, name="xt0", tag="xt", bufs=2)
        nc.sync.dma_start(out=wk_all[:, 0:8, :], in_=wk[:, 0:8, :])
        nc.scalar.dma_start(out=xts[0][:, 0:2, :], in_=xT[:, 0:2, 0:IB])
        nc.sync.dma_start(out=xts[0][:, 2:4, :], in_=xT[:, 2:4, 0:IB])
        nc.scalar.dma_start(out=xts[0][:, 4:6, :], in_=xT[:, 4:6, 0:IB])
        nc.sync.dma_start(out=xts[0][:, 6:8, :], in_=xT[:, 6:8, 0:IB])
        nc.scalar.dma_start(out=wk_all[:, 8:16, :], in_=wk[:, 8:16, :])
        nc.sync.dma_start(out=xts[0][:, 8:12, :], in_=xT[:, 8:12, 0:IB])
        nc.scalar.dma_start(out=xts[0][:, 12:16, :], in_=xT[:, 12:16, 0:IB])
        wv_all = weights.tile([P, NCT, HD], BF16, name="wv_all")
        wq_all = weights.tile([P, NCT, E], BF16, name="wq_all")
        wo_all = weights.tile([P, REP, D], BF16, name="wo_all")
        with tc.tile_wait_until(0.005):
            nc.sync.dma_start(out=wv_all[:, :, :], in_=wv[:, :, :])
        with tc.tile_wait_until(0.008):
            nc.sync.dma_start(out=wq_all[:, 0:8, :], in_=wq[:, 0:8, :])
            nc.sync.dma_start(out=wq_all[:, 8:16, :], in_=wq[:, 8:16, :])
        with tc.tile_wait_until(0.011):
            t1 = work.tile([P, NCT, IB], BF16, name="xt1", tag="xt", bufs=2)
            nc.sync.dma_start(out=t1[:, 0:8, :], in_=xT[:, 0:8, IB : 2 * IB])
            nc.sync.dma_start(out=t1[:, 8:16, :], in_=xT[:, 8:16, IB : 2 * IB])
            xts[1] = t1
        with tc.tile_wait_until(0.014):
            nc.sync.dma_start(out=wo_all[:, 0:2, :], in_=wo[:, 0:2, :])
            nc.sync.dma_start(out=wo_all[:, 2:4, :], in_=wo[:, 2:4, :])

        def load_xt(k):
            t = work.tile([P, NCT, IB], BF16, name=f"xt{k}", tag="xt", bufs=2)
            isl = slice(k * IB, (k + 1) * IB)
            nc.sync.dma_start(out=t[:, 0:8, :], in_=xT[:, 0:8, isl])
            nc.sync.dma_start(out=t[:, 8:16, :], in_=xT[:, 8:16, isl])
            xts[k] = t

        # ---- constants ----
        # sel_ones[r]: [128,128] bf16, column r all ones (den matmul lhsT).
        sel_ones = []
        for r in range(REP):
            t = persist.tile([P, P], BF16, name=f"selo{r}", tag="selo", bufs=REP)
            nc.vector.memset(t, 0.0)
            nc.vector.memset(t[:, r : r + 1], 1.0)
            sel_ones.append(t)
        # sel4[r]: [128,128] bf16, row r all ones (reciprocal broadcast lhsT).
        sel4 = []
        for r in range(REP):
            t = persist.tile([P, P], BF16, name=f"sel4{r}", tag="sel4", bufs=REP)
            nc.vector.memset(t, 1.0)
            nc.gpsimd.affine_select(
                out=t,
                in_=t,
                compare_op=mybir.AluOpType.is_equal,
                fill=0.0,
                base=-r,
                pattern=[[0, P]],
                channel_multiplier=1,
            )
            sel4.append(t)

        rec4b = persist.tile([P, IB], BF16, name="rec4b")  # rows 0:4 live
        nc.vector.memset(rec4b, 0.0)
        kT = persist.tile([P, N], BF16)  # [d, i]
        v_sb = [
            persist.tile([P, HD], BF16, name=f"v{jt}", tag="v", bufs=NJT)
            for jt in range(NJT)
        ]

        def projKV_items(ib, xt_all):
            """K/V projections for block ib (needed only from its diagonal
            groups on); yields every few matmuls."""
            isl = slice(ib * IB, (ib + 1) * IB)
            psk = psum_pp.tile([P, IB], F32, name=f"psk{ib}", tag="pp")
            for ct in range(NCT):
                nc.tensor.matmul(
                    psk, lhsT=wk_all[:, ct, :], rhs=xt_all[:, ct, :],
                    start=(ct == 0), stop=(ct == NCT - 1),
                    skip_group_check=True,
                )
                if ct % 4 == 3:
                    yield
            nc.scalar.copy(kT[:, isl], psk)
            yield
            # V directly in natural [j, d] layout: lhsT = a 128-query strip
            # of xT (contraction on partitions), rhs = Wv tile. No transpose.
            for sub in range(IB // P):
                jt = ib * (IB // P) + sub
                psv = psum_pp.tile([P, IB], F32, name=f"psv{jt}", tag="pp")
                for ct in range(NCT):
                    nc.tensor.matmul(
                        psv[:, 0:HD],
                        lhsT=xt_all[:, ct, sub * P : (sub + 1) * P],
                        rhs=wv_all[:, ct, :],
                        start=(ct == 0), stop=(ct == NCT - 1),
                        skip_group_check=True,
                    )
                    if ct % 8 == 7:
                        yield
                nc.vector.tensor_copy(v_sb[jt], psv[:, 0:HD])
                yield

        def projQ_items(ib, xt_all, qts_out):
            """Q projections for block ib; must complete before its
            attention starts."""
            for r in range(REP):
                psq = psum_pp.tile([P, IB], F32, name=f"psq{ib}_{r}", tag="pp")
                for ct in range(NCT):
                    nc.tensor.matmul(
                        psq,
                        lhsT=wq_all[:, ct, r * P : (r + 1) * P],
                        rhs=xt_all[:, ct, :],
                        start=(ct == 0), stop=(ct == NCT - 1),
                        skip_group_check=True,
                    )
                    if ct % 4 == 3:
                        yield
                qt = work.tile([P, IB], BF16, name=f"qT{ib}_{r}", tag="qT", bufs=8)
                if r % 2 == 0:
                    nc.scalar.copy(qt, psq)
                else:
                    nc.vector.tensor_copy(qt, psq)
                qts_out.append(qt)
                yield

        def outproj_items(ib, cns):
            """Output projection for block ib; yields after each 4-MM chain
            so it can serve as attention filler for block ib+1."""
            for sub in range(IB // P):
                it = ib * (IB // P) + sub
                ssl = slice(sub * P, (sub + 1) * P)
                for half in range(2):
                    o2 = work.tile(
                        [P, 2 * IB], BF16, name=f"o{it}_{half}", tag="osb",
                        bufs=4,
                    )
                    for k in range(2):
                        ot = 2 * half + k
                        pso = psum_pp.tile(
                            [P, IB], F32, name=f"pso{it}_{ot}", tag="pp"
                        )
                        for r in range(REP):
                            nc.tensor.matmul(
                                pso,
                                lhsT=cns[r][:, ssl],
                                rhs=wo_all[:, r, ot * IB : (ot + 1) * IB],
                                start=(r == 0), stop=(r == REP - 1),
                                skip_group_check=True,
                            )
                        if (it + ot) % 2 == 0:
                            nc.vector.tensor_copy(
                                o2[:, k * IB : (k + 1) * IB], pso
                            )
                        else:
                            nc.scalar.copy(o2[:, k * IB : (k + 1) * IB], pso)
                        yield
                    nc.sync.dma_start(
                        out=out[
                            it * P : (it + 1) * P,
                            half * 2 * IB : (half + 1) * 2 * IB,
                        ],
                        in_=o2,
                    )

        def drain(gen):
            for _ in gen:
                pass

        def warmup_items(n):
            for i in range(n):
                psw = psum_pp.tile([P, IB], F32, name=f"warmC{i}", tag="pp")
                nc.tensor.matmul(
                    psw, lhsT=warm[:, 0:P], rhs=warm, start=True, stop=True,
                    skip_group_check=True,
                )
                yield

        # ---- prologue: block 0 projections, warmup MMs pad DMA waits ----
        qts_cur = []
        drain(
            roundrobin(
                warmup_items(16),
                chain(projKV_items(0, xts[0]), projQ_items(0, xts[0], qts_cur)),
            )
        )

        op_gen = None  # outproj of the previous block, used as filler
        for ib in range(NBLK):
            if ib + 2 < NBLK:
                load_xt(ib + 2)
            qts_next = []
            # Filler for this block's attention: K/V of THIS block first
            # (needed by its diagonal groups, so it gets absolute priority),
            # then outproj of the previous block round-robined with Q (and
            # possibly K/V) of the next block. K/V of block ib+1 is withheld
            # from block ib when it can instead feed block ib+1's own
            # attention (keeps the last block's PE fed).
            others = []
            if ib + 1 < NBLK:
                nxt = [projQ_items(ib + 1, xts[ib + 1], qts_next)]
                if ib + 1 < 2:
                    nxt.insert(0, projKV_items(ib + 1, xts[ib + 1]))
                others.append(chain(*nxt))
            if op_gen is not None:
                others.append(op_gen)
            gen = roundrobin(*others) if others else iter(())
            if ib >= 2:
                gen = chain(projKV_items(ib, xts[ib]), gen)

            def fill(n):
                for _ in range(n):
                    next(gen, None)

            # ============ attention for this query block ============
            njt = (ib + 1) * (IB // P)  # causal: key tiles 0..njt-1
            # last diag group(s) feed the denominator via direct PE matmuls
            # (skips the DVE accumulator on the block's critical tail)
            direct_from = 2 if ib == NBLK - 1 else 3
            ps_ctx = psum_cb.tile([P, REP, IB], F32, name=f"ctx{ib}", tag="cb")
            acc = work.tile([P, REP, IB], F32, name=f"acc{ib}", tag="acc", bufs=1)
            den = None
            direct_ex = []
            for jk in range(njt):
                m = jk - (njt - 4)  # >= 0 on the diagonal strip
                i0 = max(m, 0) * P  # live columns: i >= 128*m
                ex = work.tile(
                    [P, REP, IB], BF16, name=f"ex{ib}_{jk}", tag="ex", bufs=4
                )
                for h in (0, 1):
                    sb = psum_sb.tile(
                        [P, 2, IB], F32, name=f"sb{ib}_{jk}_{h}", tag="sb"
                    )
                    for rr in (0, 1):
                        r = 2 * h + rr
                        nc.tensor.matmul(
                            sb[:, rr, i0:],
                            lhsT=kT[:, jk * P : (jk + 1) * P],
                            rhs=qts_cur[r][:, i0:],
                            start=True, stop=True, skip_group_check=True,
                        )
                    nc.scalar.activation(
                        ex[:, 2 * h : 2 * h + 2, i0:],
                        sb[:, :, i0:],
                        mybir.ActivationFunctionType.Exp,
                        scale=SCALE,
                    )
                    if m >= 0:
                        for rr in (0, 1):
                            r = 2 * h + rr
                            # triangular strip: keep where (i - i0) - j >= 0
                            nc.gpsimd.affine_select(
                                out=ex[:, r, i0 : i0 + P],
                                in_=ex[:, r, i0 : i0 + P],
                                compare_op=mybir.AluOpType.is_ge,
                                fill=0.0,
                                base=0,
                                pattern=[[1, P]],
                                channel_multiplier=-1,
                            )
                    # independent matmuls cover the scores->exp->ctx latency
                    fill(1)
                    for rr in (0, 1):
                        r = 2 * h + rr
                        nc.tensor.matmul(
                            ps_ctx[:, r, i0:],
                            lhsT=v_sb[jk],
                            rhs=ex[:, r, i0:],
                            start=(jk == 0), stop=(jk == njt - 1),
                            skip_group_check=True,
                        )
                    fill(1)
                if m < direct_from:
                    # full groups: accumulate on the mostly-idle Pool engine
                    # (DVE has the cn muls/casts; ACT paces the exp stream);
                    # diag groups keep DVE since Pool runs the affine_selects.
                    eng = nc.gpsimd if m < 0 else nc.vector
                    if jk == 0:
                        eng.tensor_copy(acc, ex)
                    else:
                        eng.tensor_add(
                            acc[:, :, i0:], acc[:, :, i0:], ex[:, :, i0:]
                        )
                else:
                    direct_ex.append((ex, i0))
                if m == direct_from - 1:
                    # denominator from the accumulator; diag tail comes from
                    # direct matmuls on the remaining ex tiles below.
                    acc_b = work.tile(
                        [P, REP, IB], BF16, name=f"accb{ib}", tag="accb", bufs=1
                    )
                    nc.vector.tensor_copy(acc_b, acc)
                    den = psum_pp.tile([P, IB], F32, name=f"den{ib}", tag="pp")
                    for r in range(REP):
                        nc.tensor.matmul(
                            den,
                            lhsT=sel_ones[r],
                            rhs=acc_b[:, r, :],
                            start=(r == 0), stop=False,
                            skip_group_check=True,
                        )
                if m >= direct_from:
                    last = m == 3
                    ex_t, exi0 = direct_ex[-1]
                    for r in range(REP):
                        nc.tensor.matmul(
                            den[:, exi0:],
                            lhsT=sel_ones[r],
                            rhs=ex_t[:, r, exi0:],
                            start=False, stop=(last and r == REP - 1),
                            skip_group_check=True,
                        )

            # ============ normalize: recip + broadcast ============
            rec4 = work.tile([REP, IB], F32, name=f"rec4{ib}", tag="rec4", bufs=2)
            nc.vector.reciprocal_approx_fast(out=rec4, in_=den[0:REP, :])
            nc.vector.tensor_copy(rec4b[0:REP, :], rec4)
            fill(2)
            cns = []
            for h in (0, 1):
                rbp = psum_sb.tile([P, 2, IB], F32, name=f"rb{ib}_{h}", tag="sb")
                for rr in (0, 1):
                    nc.tensor.matmul(
                        rbp[:, rr, :], lhsT=sel4[2 * h + rr], rhs=rec4b,
                        start=True, stop=True, skip_group_check=True,
                    )
                rbs = work.tile(
                    [P, 2, IB], BF16, name=f"rbs{ib}_{h}", tag="rbs", bufs=2
                )
                nc.scalar.copy(rbs, rbp)
                fill(2)
                for rr in (0, 1):
                    r = 2 * h + rr
                    cn = work.tile(
                        [P, IB], BF16, name=f"cn{ib}_{r}", tag="cn", bufs=8
                    )
                    nc.vector.tensor_mul(cn, ps_ctx[:, r, :], rbs[:, rr, :])
                    cns.append(cn)
            drain(gen)
            op_gen = outproj_items(ib, cns)
            qts_cur = qts_next

        drain(op_gen)


_NC_CACHE = None


def kernel(x, Wq, Wk, Wv, Wo, bo):
    global _LAST_RESULT, _NC_CACHE
    x = np.asarray(x, dtype=np.float32)
    Wq = np.asarray(Wq, dtype=np.float32)
    Wk = np.asarray(Wk, dtype=np.float32)
    Wv = np.asarray(Wv, dtype=np.float32)
    Wo = np.asarray(Wo, dtype=np.float32)
    bo = np.asarray(bo, dtype=np.float32)

    if _NC_CACHE is None:
        _NC_CACHE = build_bass()
    nc = _NC_CACHE

    def chunked(a, pdim):
        # [pdim*nchunk, F] -> [pdim, nchunk, F] bf16, partition-major
        nchunk = a.shape[0] // pdim
        return np.ascontiguousarray(
            a.reshape(nchunk, pdim, a.shape[1]).transpose(1, 0, 2)
        ).astype(ml_dtypes.bfloat16)

    in_maps = []
    for core in range(8):
        b, g = core // G, core % G
        in_maps.append(
            {
                "xT": chunked(np.ascontiguousarray(x[b].T), P),
                "wq": chunked(Wq[:, g * E : (g + 1) * E], P),
                "wk": chunked(Wk[:, g * HD : (g + 1) * HD], P),
                "wv": chunked(Wv[:, g * HD : (g + 1) * HD], P),
                "wo": chunked(Wo[g * E : (g + 1) * E, :], P),
            }
        )
    res = run_bass_kernel_spmd(
        nc,
        in_maps,
        list(range(8)),
        trace=bool(os.environ.get("BASS_TRACE")),
    )
    _LAST_RESULT = res
    partials = np.stack(
        [np.asarray(r["out"]).astype(np.float32) for r in res.results]
    )  # [8, N, D]
    full = partials.reshape(B, G, N, D).sum(axis=1) + bo[None, None, :]
    return full.astype(np.float32)


# revision 10
# speedup vs baseline: 1.1953x; 1.0248x over previous
"""GQA attention kernel for Trainium2, 8 NeuronCores.

Problem: x[2,2048,2048] @ Wq/Wk/Wv -> grouped-query attention (16 q heads,
4 kv groups, head_dim 128, causal) -> @ Wo + bo.

Sharding: (batch b in 0..1) x (kv group g in 0..3) -> 8 cores.
Each core computes the full attention for its (b, g): 4 query heads sharing
one kv head, then a row-parallel partial of the output projection
(ctx_g @ Wo[g*512:(g+1)*512, :]). Host sums the 4 group partials per batch
and adds the bias.

Perf design (282us baseline for this file's ancestor -> this version):
  - softmax denominator accumulated on DVE (acc += ex per key tile) instead
    of a PE matmul per (r, jk); only a few sel_ones matmuls per block on the
    final accumulator (-26us of PE streaming).
  - exp merged across head pairs: scores for (r, r+1) land in one 2-bank
    PSUM tile, one ACT instruction exponentiates [128, 2, w] (fixed ~260ns
    ACT overhead amortized; ACT paces the attention inner loop).
  - the in-order PE queue is never left waiting on ACT: independent matmuls
    (outproj of block ib-1, then projections of block ib+1) are emitted in
    ~4-MM chunks BETWEEN each exp and its dependent ctx matmuls. Outproj
    lagging one block is what gives the final block filler too - otherwise
    the PE goes sparse and the HAM clock-gate drops it to 1.2 GHz.
  - warmup matmuls on a zeroed tile run during the initial DMA wait so HAM
    un-throttles before the first real matmul.
  - startup-critical DMAs (wk, x block 0) get the DMA rings to themselves;
    wv/wq/wo/x1 are held back ~5-14us via tile_wait_until.
  - output written bf16 (host sums partials in f32): halves out DMA.
  - all matmul inputs bf16; every stationary operand 128x128.
"""

import os
from itertools import chain

import ml_dtypes
import numpy as np

import concourse.bass as bass
from concourse import bacc
import concourse.bass_isa as bass_isa
import concourse.mybir as mybir
from concourse.bass_utils import run_bass_kernel_spmd
from concourse.tile import TileContext

B, N, D = 2, 2048, 2048
G, REP, HD = 4, 4, 128
E = REP * HD  # 512 q-dims per group
P = 128
IB = 512  # i-block (query block) size
NBLK = N // IB  # 4
NCT = D // P  # 16 contraction tiles
NJT = N // P  # 16 key tiles
SCALE = 1.0 / float(np.sqrt(HD))

F32 = mybir.dt.float32
BF16 = mybir.dt.bfloat16

NWARM_512 = 6
NWARM_128 = 12

_LAST_RESULT = None  # test.py reads exec_time_ns from here


def build_bass():
    nc = bacc.Bacc()
    # All inputs bf16, laid out [partition, chunk, free] on the host so each
    # tensor needs few DMA descriptors.
    xT = nc.dram_tensor("xT", [P, NCT, N], BF16, kind="ExternalInput")
    wq = nc.dram_tensor("wq", [P, NCT, E], BF16, kind="ExternalInput")
    wk = nc.dram_tensor("wk", [P, NCT, HD], BF16, kind="ExternalInput")
    wv = nc.dram_tensor("wv", [P, NCT, HD], BF16, kind="ExternalInput")
    wo = nc.dram_tensor("wo", [P, REP, D], BF16, kind="ExternalInput")
    out = nc.dram_tensor("out", [N, D], BF16, kind="ExternalOutput")

    with TileContext(nc) as tc:
        build_tile_kernel(nc, tc, xT, wq, wk, wv, wo, out)
    nc.finalize()
    return nc


def roundrobin(*gens):
    gens = [g for g in gens if g is not None]
    while gens:
        nxt = []
        for g in gens:
            try:
                next(g)
            except StopIteration:
                continue
            nxt.append(g)
            yield
        gens = nxt


def build_tile_kernel(nc, tc, xT, wq, wk, wv, wo, out):
    import contextlib

    ctx = contextlib.ExitStack()
    with ctx:
        persist = ctx.enter_context(tc.tile_pool(name="persist", bufs=1))
        weights = ctx.enter_context(tc.tile_pool(name="weights", bufs=1))
        work = ctx.enter_context(tc.tile_pool(name="work", bufs=2))
        # PSUM budget (8 banks): scores 2 + ctx 4 + proj/outproj 2
        psum_sb = ctx.enter_context(
            tc.tile_pool(name="psum_sb", bufs=1, space="PSUM")
        )
        psum_cb = ctx.enter_context(
            tc.tile_pool(name="psum_cb", bufs=1, space="PSUM")
        )
        psum_pp = ctx.enter_context(
            tc.tile_pool(name="psum_pp", bufs=2, space="PSUM")
        )

        # ---- warmup: matmuls on zeroed data while input DMAs fly ----
        warm = persist.tile([P, IB], BF16, name="warm")
        nc.vector.memset(warm, 0.0)
        for i in range(NWARM_512):
            psw = psum_pp.tile([P, IB], F32, name=f"warmA{i}", tag="pp")
            nc.tensor.matmul(
                psw, lhsT=warm[:, 0:P], rhs=warm, start=True, stop=True,
                skip_group_check=True,
            )
        for i in range(NWARM_128):
            psw = psum_pp.tile([P, IB], F32, name=f"warmB{i}", tag="pp")
            nc.tensor.matmul(
                psw[:, 0:P], lhsT=warm[:, 0:P], rhs=warm[:, 0:P],
                start=True, stop=True, skip_group_check=True,
            )

        # ---- startup DMAs ----
        # Criticals first (wk + x block 0) with the rings to themselves;
        # the rest is staggered so it doesn't steal bandwidth from them.
        wk_all = weights.tile([P, NCT, HD], BF16, name="wk_all")
        xts = {}
        xts[0] = work.tile([P, NCT, IB], BF16, name="xt0", tag="xt", bufs=2)
        nc.sync.dma_start(out=wk_all[:, 0:8, :], in_=wk[:, 0:8, :])
        nc.scalar.dma_start(out=xts[0][:, 0:2, :], in_=xT[:, 0:2, 0:IB])
        nc.sync.dma_start(out=xts[0][:, 2:4, :], in_=xT[:, 2:4, 0:IB])
        nc.scalar.dma_start(out=xts[0][:, 4:6, :], in_=xT[:, 4:6, 0:IB])
        nc.sync.dma_start(out=xts[0][:, 6:8, :], in_=xT[:, 6:8, 0:IB])
        nc.scalar.dma_start(out=wk_all[:, 8:16, :], in_=wk[:, 8:16, :])
        nc.sync.dma_start(out=xts[0][:, 8:12, :], in_=xT[:, 8:12, 0:IB])
        nc.scalar.dma_start(out=xts[0][:, 12:16, :], in_=xT[:, 12:16, 0:IB])
        wv_all = weights.tile([P, NCT, HD], BF16, name="wv_all")
        wq_all = weights.tile([P, NCT, E], BF16, name="wq_all")
        wo_all = weights.tile([P, REP, D], BF16, name="wo_all")
        with tc.tile_wait_until(0.005):
            nc.sync.dma_start(out=wv_all[:, :, :], in_=wv[:, :, :])
        with tc.tile_wait_until(0.008):
            nc.sync.dma_start(out=wq_all[:, 0:8, :], in_=wq[:, 0:8, :])
            nc.sync.dma_start(out=wq_all[:, 8:16, :], in_=wq[:, 8:16, :])
        with tc.tile_wait_until(0.011):
            t1 = work.tile([P, NCT, IB], BF16, name="xt1", tag="xt", bufs=2)
            nc.sync.dma_start(out=t1[:, 0:8, :], in_=xT[:, 0:8, IB : 2 * IB])
            nc.sync.dma_start(out=t1[:, 8:16, :], in_=xT[:, 8:16, IB : 2 * IB])
            xts[1] = t1
        with tc.tile_wait_until(0.014):
            nc.sync.dma_start(out=wo_all[:, 0:2, :], in_=wo[:, 0:2, :])
            nc.sync.dma_start(out=wo_all[:, 2:4, :], in_=wo[:, 2:4, :])

        def load_xt(k):
            t = work.tile([P, NCT, IB], BF16, name=f"xt{k}", tag="xt", bufs=2)
            isl = slice(k * IB, (k + 1) * IB)
            nc.sync.dma_start(out=t[:, 0:8, :], in_=xT[:, 0:8, isl])
            nc.sync.dma_start(out=t[:, 8:16, :], in_=xT[:, 8:16, isl])
            xts[k] = t

        # ---- constants ----
        # sel_ones[r]: [128,128] bf16, column r all ones (den matmul lhsT).
        sel_ones = []
        for r in range(REP):
            t = persist.tile([P, P], BF16, name=f"selo{r}", tag="selo", bufs=REP)
            nc.vector.memset(t, 0.0)
            nc.vector.memset(t[:, r : r + 1], 1.0)
            sel_ones.append(t)
        # sel4[r]: [128,128] bf16, row r all ones (reciprocal broadcast lhsT).
        sel4 = []
        for r in range(REP):
            t = persist.tile([P, P], BF16, name=f"sel4{r}", tag="sel4", bufs=REP)
            nc.vector.memset(t, 1.0)
            nc.gpsimd.affine_select(
                out=t,
                in_=t,
                compare_op=mybir.AluOpType.is_equal,
                fill=0.0,
                base=-r,
                pattern=[[0, P]],
                channel_multiplier=1,
            )
            sel4.append(t)

        rec4b = persist.tile([P, IB], BF16, name="rec4b")  # rows 0:4 live
        nc.vector.memset(rec4b, 0.0)
        kT = persist.tile([P, N], BF16)  # [d, i]
        v_sb = [
            persist.tile([P, HD], BF16, name=f"v{jt}", tag="v", bufs=NJT)
            for jt in range(NJT)
        ]

        def projKV_items(ib, xt_all):
            """K/V projections for block ib (needed only from its diagonal
            groups on); yields every few matmuls."""
            isl = slice(ib * IB, (ib + 1) * IB)
            psk = psum_pp.tile([P, IB], F32, name=f"psk{ib}", tag="pp")
            for ct in range(NCT):
                nc.tensor.matmul(
                    psk, lhsT=wk_all[:, ct, :], rhs=xt_all[:, ct, :],
                    start=(ct == 0), stop=(ct == NCT - 1),
                    skip_group_check=True,
                )
                if ct % 4 == 3:
                    yield
            nc.scalar.copy(kT[:, isl], psk)
            yield
            # V directly in natural [j, d] layout: lhsT = a 128-query strip
            # of xT (contraction on partitions), rhs = Wv tile. No transpose.
            for sub in range(IB // P):
                jt = ib * (IB // P) + sub
                psv = psum_pp.tile([P, IB], F32, name=f"psv{jt}", tag="pp")
                for ct in range(NCT):
                    nc.tensor.matmul(
                        psv[:, 0:HD],
                        lhsT=xt_all[:, ct, sub * P : (sub + 1) * P],
                        rhs=wv_all[:, ct, :],
                        start=(ct == 0), stop=(ct == NCT - 1),
                        skip_group_check=True,
                    )
                    if ct % 8 == 7:
                        yield
                nc.vector.tensor_copy(v_sb[jt], psv[:, 0:HD])
                yield

        def projQ_items(ib, xt_all, qts_out):
            """Q projections for block ib; must complete before its
            attention starts."""
            for r in range(REP):
                psq = psum_pp.tile([P, IB], F32, name=f"psq{ib}_{r}", tag="pp")
                for ct in range(NCT):
                    nc.tensor.matmul(
                        psq,
                        lhsT=wq_all[:, ct, r * P : (r + 1) * P],
                        rhs=xt_all[:, ct, :],
                        start=(ct == 0), stop=(ct == NCT - 1),
                        skip_group_check=True,
                    )
                    if ct % 4 == 3:
                        yield
                qt = work.tile([P, IB], BF16, name=f"qT{ib}_{r}", tag="qT", bufs=8)
                if r % 2 == 0:
                    nc.scalar.copy(qt, psq)
                else:
                    nc.vector.tensor_copy(qt, psq)
                qts_out.append(qt)
                yield

        def outproj_items(ib, cns):
            """Output projection for block ib; yields after each 4-MM chain
            so it can serve as attention filler for block ib+1."""
            for sub in range(IB // P):
                it = ib * (IB // P) + sub
                ssl = slice(sub * P, (sub + 1) * P)
                for half in range(2):
                    o2 = work.tile(
                        [P, 2 * IB], BF16, name=f"o{it}_{half}", tag="osb",
                        bufs=4,
                    )
                    for k in range(2):
                        ot = 2 * half + k
                        pso = psum_pp.tile(
                            [P, IB], F32, name=f"pso{it}_{ot}", tag="pp"
                        )
                        for r in range(REP):
                            nc.tensor.matmul(
                                pso,
                                lhsT=cns[r][:, ssl],
                                rhs=wo_all[:, r, ot * IB : (ot + 1) * IB],
                                start=(r == 0), stop=(r == REP - 1),
                                skip_group_check=True,
                            )
                        if (it + ot) % 2 == 0:
                            nc.vector.tensor_copy(
                                o2[:, k * IB : (k + 1) * IB], pso
                            )
                        else:
                            nc.scalar.copy(o2[:, k * IB : (k + 1) * IB], pso)
                        yield
                    nc.sync.dma_start(
                        out=out[
                            it * P : (it + 1) * P,
                            half * 2 * IB : (half + 1) * 2 * IB,
                        ],
                        in_=o2,
                    )

        def drain(gen):
            for _ in gen:
                pass

        def warmup_items(n):
            for i in range(n):
                psw = psum_pp.tile([P, IB], F32, name=f"warmC{i}", tag="pp")
                nc.tensor.matmul(
                    psw, lhsT=warm[:, 0:P], rhs=warm, start=True, stop=True,
                    skip_group_check=True,
                )
                yield

        # ---- prologue: block 0 projections, warmup MMs pad DMA waits ----
        qts_cur = []
        drain(
            roundrobin(
                warmup_items(16),
                chain(projKV_items(0, xts[0]), projQ_items(0, xts[0], qts_cur)),
            )
        )

        op_gen = None  # outproj of the previous block, used as filler
        for ib in range(NBLK):
            if ib + 2 < NBLK:
                load_xt(ib + 2)
            qts_next = []
            # Filler for this block's attention: K/V of THIS block first
            # (needed by its diagonal groups, so it gets absolute priority),
            # then outproj of the previous block round-robined with Q (and
            # possibly K/V) of the next block. K/V of block ib+1 is withheld
            # from block ib when it can instead feed block ib+1's own
            # attention (keeps the last block's PE fed).
            others = []
            if ib + 1 < NBLK:
                nxt = [projQ_items(ib + 1, xts[ib + 1], qts_next)]
                if ib + 1 < 2:
                    nxt.insert(0, projKV_items(ib + 1, xts[ib + 1]))
                others.append(chain(*nxt))
            if op_gen is not None:
                others.append(op_gen)
            gen = roundrobin(*others) if others else iter(())
            if ib >= 2:
                gen = chain(projKV_items(ib, xts[ib]), gen)

            def fill(n):
                for _ in range(n):
                    next(gen, None)

            # ============ attention for this query block ============
            njt = (ib + 1) * (IB // P)  # causal: key tiles 0..njt-1
            # last diag group(s) feed the denominator via direct PE matmuls
            # (skips the DVE accumulator on the block's critical tail)
            direct_from = 2 if ib == NBLK - 1 else 3
            ps_ctx = psum_cb.tile([P, REP, IB], F32, name=f"ctx{ib}", tag="cb")
            acc = work.tile([P, REP, IB], F32, name=f"acc{ib}", tag="acc", bufs=1)
            den = None
            direct_ex = []
            for jk in range(njt):
                m = jk - (njt - 4)  # >= 0 on the diagonal strip
                i0 = max(m, 0) * P  # live columns: i >= 128*m
                ex = work.tile(
                    [P, REP, IB], BF16, name=f"ex{ib}_{jk}", tag="ex", bufs=4
                )
                for h in (0, 1):
                    sb = psum_sb.tile(
                        [P, 2, IB], F32, name=f"sb{ib}_{jk}_{h}", tag="sb"
                    )
                    for rr in (0, 1):
                        r = 2 * h + rr
                        nc.tensor.matmul(
                            sb[:, rr, i0:],
                            lhsT=kT[:, jk * P : (jk + 1) * P],
                            rhs=qts_cur[r][:, i0:],
                            start=True, stop=True, skip_group_check=True,
                        )
                    nc.scalar.activation(
                        ex[:, 2 * h : 2 * h + 2, i0:],
                        sb[:, :, i0:],
                        mybir.ActivationFunctionType.Exp,
                        scale=SCALE,
                    )
                    if m >= 0:
                        for rr in (0, 1):
                            r = 2 * h + rr
                            # triangular strip: keep where (i - i0) - j >= 0
                            nc.gpsimd.affine_select(
                                out=ex[:, r, i0 : i0 + P],
                                in_=ex[:, r, i0 : i0 + P],
                                compare_op=mybir.AluOpType.is_ge,
                                fill=0.0,
                                base=0,
                                pattern=[[1, P]],
                                channel_multiplier=-1,
                            )
                    # independent matmuls cover the scores->exp->ctx latency
                    fill(1)
                    for rr in (0, 1):
                        r = 2 * h + rr
                        nc.tensor.matmul(
                            ps_ctx[:, r, i0:],
                            lhsT=v_sb[jk],
                            rhs=ex[:, r, i0:],
                            start=(jk == 0), stop=(jk == njt - 1),
                            skip_group_check=True,
                        )
                    fill(1)
                if m < direct_from:
                    # DVE owns the accumulator (Pool measured 2x slower per
                    # add and the chain is serial per engine).
                    if jk == 0:
                        nc.vector.tensor_copy(acc, ex)
                    else:
                        nc.vector.tensor_add(
                            acc[:, :, i0:], acc[:, :, i0:], ex[:, :, i0:]
                        )
                else:
                    direct_ex.append((ex, i0))
                if m == direct_from - 1:
                    # denominator from the accumulator; diag tail comes from
                    # direct matmuls on the remaining ex tiles below.
                    acc_b = work.tile(
                        [P, REP, IB], BF16, name=f"accb{ib}", tag="accb", bufs=1
                    )
                    nc.vector.tensor_copy(acc_b, acc)
                    den = psum_pp.tile([P, IB], F32, name=f"den{ib}", tag="pp")
                    for r in range(REP):
                        nc.tensor.matmul(
                            den,
                            lhsT=sel_ones[r],
                            rhs=acc_b[:, r, :],
                            start=(r == 0), stop=False,
                            skip_group_check=True,
                        )
                if m >= direct_from:
                    last = m == 3
                    ex_t, exi0 = direct_ex[-1]
                    for r in range(REP):
                        nc.tensor.matmul(
                            den[:, exi0:],
                            lhsT=sel_ones[r],
                            rhs=ex_t[:, r, exi0:],
                            start=False, stop=(last and r == REP - 1),
                            skip_group_check=True,
                        )

            # ============ normalize: recip + broadcast ============
            rec4 = work.tile([REP, IB], F32, name=f"rec4{ib}", tag="rec4", bufs=2)
            nc.vector.reciprocal_approx_fast(out=rec4, in_=den[0:REP, :])
            nc.vector.tensor_copy(rec4b[0:REP, :], rec4)
            fill(2)
            cns = []
            for h in (0, 1):
                rbp = psum_sb.tile([P, 2, IB], F32, name=f"rb{ib}_{h}", tag="sb")
                for rr in (0, 1):
                    nc.tensor.matmul(
                        rbp[:, rr, :], lhsT=sel4[2 * h + rr], rhs=rec4b,
                        start=True, stop=True, skip_group_check=True,
                    )
                rbs = work.tile(
                    [P, 2, IB], BF16, name=f"rbs{ib}_{h}", tag="rbs", bufs=2
                )
                nc.scalar.copy(rbs, rbp)
                fill(2)
                for rr in (0, 1):
                    r = 2 * h + rr
                    cn = work.tile(
                        [P, IB], BF16, name=f"cn{ib}_{r}", tag="cn", bufs=8
                    )
                    nc.vector.tensor_mul(cn, ps_ctx[:, r, :], rbs[:, rr, :])
                    cns.append(cn)
            drain(gen)
            op_gen = outproj_items(ib, cns)
            qts_cur = qts_next

        drain(op_gen)


_NC_CACHE = None


def kernel(x, Wq, Wk, Wv, Wo, bo):
    global _LAST_RESULT, _NC_CACHE
    x = np.asarray(x, dtype=np.float32)
    Wq = np.asarray(Wq, dtype=np.float32)
    Wk = np.asarray(Wk, dtype=np.float32)
    Wv = np.asarray(Wv, dtype=np.float32)
    Wo = np.asarray(Wo, dtype=np.float32)
    bo = np.asarray(bo, dtype=np.float32)

    if _NC_CACHE is None:
        _NC_CACHE = build_bass()
    nc = _NC_CACHE

    def chunked(a, pdim):
        # [pdim*nchunk, F] -> [pdim, nchunk, F] bf16, partition-major
        nchunk = a.shape[0] // pdim
        return np.ascontiguousarray(
            a.reshape(nchunk, pdim, a.shape[1]).transpose(1, 0, 2)
        ).astype(ml_dtypes.bfloat16)

    in_maps = []
    for core in range(8):
        b, g = core // G, core % G
        in_maps.append(
            {
                "xT": chunked(np.ascontiguousarray(x[b].T), P),
                "wq": chunked(Wq[:, g * E : (g + 1) * E], P),
                "wk": chunked(Wk[:, g * HD : (g + 1) * HD], P),
                "wv": chunked(Wv[:, g * HD : (g + 1) * HD], P),
                "wo": chunked(Wo[g * E : (g + 1) * E, :], P),
            }
        )
    res = run_bass_kernel_spmd(
        nc,
        in_maps,
        list(range(8)),
        trace=bool(os.environ.get("BASS_TRACE")),
    )
    _LAST_RESULT = res
    partials = np.stack(
        [np.asarray(r["out"]).astype(np.float32) for r in res.results]
    )  # [8, N, D]
    full = partials.reshape(B, G, N, D).sum(axis=1) + bo[None, None, :]
    return full.astype(np.float32)


# revision 13
# speedup vs baseline: 1.3282x; 1.1111x over previous
"""GQA attention kernel for Trainium2, 8 NeuronCores.

Problem: x[2,2048,2048] @ Wq/Wk/Wv -> grouped-query attention (16 q heads,
4 kv groups, head_dim 128, causal) -> @ Wo + bo.

Sharding: (batch b in 0..1) x (kv group g in 0..3) -> 8 cores.
Each core computes the full attention for its (b, g): 4 query heads sharing
one kv head, then a row-parallel partial of the output projection
(ctx_g @ Wo[g*512:(g+1)*512, :]). Host sums the 4 group partials per batch
and adds the bias.

Perf design (282us baseline for this file's ancestor -> this version):
  - softmax denominator accumulated on DVE (acc += ex per key tile) instead
    of a PE matmul per (r, jk); only a few sel_ones matmuls per block on the
    final accumulator (-26us of PE streaming).
  - exp merged across head pairs: scores for (r, r+1) land in one 2-bank
    PSUM tile, one ACT instruction exponentiates [128, 2, w] (fixed ~260ns
    ACT overhead amortized; ACT paces the attention inner loop).
  - the in-order PE queue is never left waiting on ACT: independent matmuls
    (outproj of block ib-1, then projections of block ib+1) are emitted in
    ~4-MM chunks BETWEEN each exp and its dependent ctx matmuls. Outproj
    lagging one block is what gives the final block filler too - otherwise
    the PE goes sparse and the HAM clock-gate drops it to 1.2 GHz.
  - warmup matmuls on a zeroed tile run during the initial DMA wait so HAM
    un-throttles before the first real matmul.
  - startup-critical DMAs (wk, x block 0) get the DMA rings to themselves;
    wv/wq/wo/x1 are held back ~5-14us via tile_wait_until.
  - output written bf16 (host sums partials in f32): halves out DMA.
  - all matmul inputs bf16; every stationary operand 128x128.
"""

import os
from itertools import chain

import ml_dtypes
import numpy as np

import concourse.bass as bass
from concourse import bacc
import concourse.bass_isa as bass_isa
import concourse.mybir as mybir
from concourse.bass_utils import run_bass_kernel_spmd
from concourse.tile import TileContext

B, N, D = 2, 2048, 2048
G, REP, HD = 4, 4, 128
E = REP * HD  # 512 q-dims per group
P = 128
IB = 512  # i-block (query block) size
NBLK = N // IB  # 4
NCT = D // P  # 16 contraction tiles
NJT = N // P  # 16 key tiles
SCALE = 1.0 / float(np.sqrt(HD))

F32 = mybir.dt.float32
BF16 = mybir.dt.bfloat16

NWARM_512 = 6
NWARM_128 = 12

_LAST_RESULT = None  # test.py reads exec_time_ns from here


def build_bass():
    nc = bacc.Bacc()
    # All inputs bf16, laid out [partition, chunk, free] on the host so each
    # tensor needs few DMA descriptors.
    xT = nc.dram_tensor("xT", [P, NCT, N], BF16, kind="ExternalInput")
    wq = nc.dram_tensor("wq", [P, NCT, E], BF16, kind="ExternalInput")
    wk = nc.dram_tensor("wk", [P, NCT, HD], BF16, kind="ExternalInput")
    wv = nc.dram_tensor("wv", [P, NCT, HD], BF16, kind="ExternalInput")
    wo = nc.dram_tensor("wo", [P, REP, D], BF16, kind="ExternalInput")
    out = nc.dram_tensor("out", [N, D], BF16, kind="ExternalOutput")

    with TileContext(nc) as tc:
        build_tile_kernel(nc, tc, xT, wq, wk, wv, wo, out)
    nc.finalize()
    return nc


def roundrobin(*gens):
    gens = [g for g in gens if g is not None]
    while gens:
        nxt = []
        for g in gens:
            try:
                next(g)
            except StopIteration:
                continue
            nxt.append(g)
            yield
        gens = nxt


def build_tile_kernel(nc, tc, xT, wq, wk, wv, wo, out):
    import contextlib

    ctx = contextlib.ExitStack()
    with ctx:
        persist = ctx.enter_context(tc.tile_pool(name="persist", bufs=1))
        weights = ctx.enter_context(tc.tile_pool(name="weights", bufs=1))
        work = ctx.enter_context(tc.tile_pool(name="work", bufs=2))
        # PSUM budget (8 banks): scores 2 + ctx 4 + proj/outproj 2
        psum_sb = ctx.enter_context(
            tc.tile_pool(name="psum_sb", bufs=1, space="PSUM")
        )
        psum_cb = ctx.enter_context(
            tc.tile_pool(name="psum_cb", bufs=1, space="PSUM")
        )
        psum_pp = ctx.enter_context(
            tc.tile_pool(name="psum_pp", bufs=2, space="PSUM")
        )

        # ---- warmup: matmuls on zeroed data while input DMAs fly ----
        warm = persist.tile([P, IB], BF16, name="warm")
        nc.vector.memset(warm, 0.0)
        for i in range(NWARM_512):
            psw = psum_pp.tile([P, IB], F32, name=f"warmA{i}", tag="pp")
            nc.tensor.matmul(
                psw, lhsT=warm[:, 0:P], rhs=warm, start=True, stop=True,
                skip_group_check=True,
            )
        for i in range(NWARM_128):
            psw = psum_pp.tile([P, IB], F32, name=f"warmB{i}", tag="pp")
            nc.tensor.matmul(
                psw[:, 0:P], lhsT=warm[:, 0:P], rhs=warm[:, 0:P],
                start=True, stop=True, skip_group_check=True,
            )

        # ---- startup DMAs ----
        # Criticals first (wk + x block 0) with the rings to themselves;
        # the rest is staggered so it doesn't steal bandwidth from them.
        wk_all = weights.tile([P, NCT, HD], BF16, name="wk_all")
        xts = {}
        xts[0] = work.tile([P, NCT, IB], BF16, name="xt0", tag="xt", bufs=2)
        nc.sync.dma_start(out=wk_all[:, 0:8, :], in_=wk[:, 0:8, :])
        nc.scalar.dma_start(out=xts[0][:, 0:2, :], in_=xT[:, 0:2, 0:IB])
        nc.sync.dma_start(out=xts[0][:, 2:4, :], in_=xT[:, 2:4, 0:IB])
        nc.scalar.dma_start(out=xts[0][:, 4:6, :], in_=xT[:, 4:6, 0:IB])
        nc.sync.dma_start(out=xts[0][:, 6:8, :], in_=xT[:, 6:8, 0:IB])
        nc.scalar.dma_start(out=wk_all[:, 8:16, :], in_=wk[:, 8:16, :])
        nc.sync.dma_start(out=xts[0][:, 8:12, :], in_=xT[:, 8:12, 0:IB])
        nc.scalar.dma_start(out=xts[0][:, 12:16, :], in_=xT[:, 12:16, 0:IB])
        wv_all = weights.tile([P, NCT, HD], BF16, name="wv_all")
        wq_all = weights.tile([P, NCT, E], BF16, name="wq_all")
        wo_all = weights.tile([P, REP, D], BF16, name="wo_all")
        with tc.tile_wait_until(0.005):
            nc.sync.dma_start(out=wv_all[:, :, :], in_=wv[:, :, :])
        with tc.tile_wait_until(0.0065):
            nc.sync.dma_start(out=wq_all[:, 0:8, :], in_=wq[:, 0:8, :])
            nc.sync.dma_start(out=wq_all[:, 8:16, :], in_=wq[:, 8:16, :])
        with tc.tile_wait_until(0.0095):
            t1 = work.tile([P, NCT, IB], BF16, name="xt1", tag="xt", bufs=2)
            nc.sync.dma_start(out=t1[:, 0:8, :], in_=xT[:, 0:8, IB : 2 * IB])
            nc.sync.dma_start(out=t1[:, 8:16, :], in_=xT[:, 8:16, IB : 2 * IB])
            xts[1] = t1
        with tc.tile_wait_until(0.012):
            nc.sync.dma_start(out=wo_all[:, 0:2, :], in_=wo[:, 0:2, :])
            nc.sync.dma_start(out=wo_all[:, 2:4, :], in_=wo[:, 2:4, :])

        def load_xt(k):
            t = work.tile([P, NCT, IB], BF16, name=f"xt{k}", tag="xt", bufs=2)
            isl = slice(k * IB, (k + 1) * IB)
            nc.sync.dma_start(out=t[:, 0:8, :], in_=xT[:, 0:8, isl])
            nc.sync.dma_start(out=t[:, 8:16, :], in_=xT[:, 8:16, isl])
            xts[k] = t

        # ---- constants ----
        # sel_ones[r]: [128,128] bf16, column r all ones (den matmul lhsT).
        sel_ones = []
        for r in range(REP):
            t = persist.tile([P, P], BF16, name=f"selo{r}", tag="selo", bufs=REP)
            nc.vector.memset(t, 0.0)
            nc.vector.memset(t[:, r : r + 1], 1.0)
            sel_ones.append(t)
        # sel4[r]: [128,128] bf16, row r all ones (reciprocal broadcast lhsT).
        sel4 = []
        for r in range(REP):
            t = persist.tile([P, P], BF16, name=f"sel4{r}", tag="sel4", bufs=REP)
            nc.vector.memset(t, 1.0)
            nc.gpsimd.affine_select(
                out=t,
                in_=t,
                compare_op=mybir.AluOpType.is_equal,
                fill=0.0,
                base=-r,
                pattern=[[0, P]],
                channel_multiplier=1,
            )
            sel4.append(t)

        rec4b = persist.tile([P, IB], BF16, name="rec4b")  # rows 0:4 live
        nc.vector.memset(rec4b, 0.0)
        kT = persist.tile([P, N], BF16)  # [d, i]
        v_sb = [
            persist.tile([P, HD], BF16, name=f"v{jt}", tag="v", bufs=NJT)
            for jt in range(NJT)
        ]

        def projKV_items(ib, xt_all):
            """K/V projections for block ib (needed only from its diagonal
            groups on); yields every few matmuls."""
            isl = slice(ib * IB, (ib + 1) * IB)
            psk = psum_pp.tile([P, IB], F32, name=f"psk{ib}", tag="pp")
            for ct in range(NCT):
                nc.tensor.matmul(
                    psk, lhsT=wk_all[:, ct, :], rhs=xt_all[:, ct, :],
                    start=(ct == 0), stop=(ct == NCT - 1),
                    skip_group_check=True,
                )
                if ct % 4 == 3:
                    yield
            nc.scalar.copy(kT[:, isl], psk)
            yield
            # V directly in natural [j, d] layout: lhsT = a 128-query strip
            # of xT (contraction on partitions), rhs = Wv tile. No transpose.
            for sub in range(IB // P):
                jt = ib * (IB // P) + sub
                psv = psum_pp.tile([P, IB], F32, name=f"psv{jt}", tag="pp")
                for ct in range(NCT):
                    nc.tensor.matmul(
                        psv[:, 0:HD],
                        lhsT=xt_all[:, ct, sub * P : (sub + 1) * P],
                        rhs=wv_all[:, ct, :],
                        start=(ct == 0), stop=(ct == NCT - 1),
                        skip_group_check=True,
                    )
                    if ct % 8 == 7:
                        yield
                nc.vector.tensor_copy(v_sb[jt], psv[:, 0:HD])
                yield

        def projQ_items(ib, xt_all, qts_out):
            """Q projections for block ib; must complete before its
            attention starts."""
            for r in range(REP):
                psq = psum_pp.tile([P, IB], F32, name=f"psq{ib}_{r}", tag="pp")
                for ct in range(NCT):
                    nc.tensor.matmul(
                        psq,
                        lhsT=wq_all[:, ct, r * P : (r + 1) * P],
                        rhs=xt_all[:, ct, :],
                        start=(ct == 0), stop=(ct == NCT - 1),
                        skip_group_check=True,
                    )
                    if ct % 4 == 3:
                        yield
                qt = work.tile([P, IB], BF16, name=f"qT{ib}_{r}", tag="qT", bufs=8)
                if r % 2 == 0:
                    nc.scalar.copy(qt, psq)
                else:
                    nc.vector.tensor_copy(qt, psq)
                qts_out.append(qt)
                yield

        def outproj_items(ib, cns):
            """Output projection for block ib; yields after each 4-MM chain
            so it can serve as attention filler for block ib+1."""
            for sub in range(IB // P):
                it = ib * (IB // P) + sub
                ssl = slice(sub * P, (sub + 1) * P)
                for half in range(2):
                    o2 = work.tile(
                        [P, 2 * IB], BF16, name=f"o{it}_{half}", tag="osb",
                        bufs=4,
                    )
                    for k in range(2):
                        ot = 2 * half + k
                        pso = psum_pp.tile(
                            [P, IB], F32, name=f"pso{it}_{ot}", tag="pp"
                        )
                        for r in range(REP):
                            nc.tensor.matmul(
                                pso,
                                lhsT=cns[r][:, ssl],
                                rhs=wo_all[:, r, ot * IB : (ot + 1) * IB],
                                start=(r == 0), stop=(r == REP - 1),
                                skip_group_check=True,
                            )
                        if (it + ot) % 2 == 0:
                            nc.vector.tensor_copy(
                                o2[:, k * IB : (k + 1) * IB], pso
                            )
                        else:
                            nc.scalar.copy(o2[:, k * IB : (k + 1) * IB], pso)
                        yield
                    nc.sync.dma_start(
                        out=out[
                            it * P : (it + 1) * P,
                            half * 2 * IB : (half + 1) * 2 * IB,
                        ],
                        in_=o2,
                    )

        def drain(gen):
            for _ in gen:
                pass

        def warmup_items(n):
            for i in range(n):
                psw = psum_pp.tile([P, IB], F32, name=f"warmC{i}", tag="pp")
                nc.tensor.matmul(
                    psw, lhsT=warm[:, 0:P], rhs=warm, start=True, stop=True,
                    skip_group_check=True,
                )
                yield

        # ---- prologue: block 0 projections, warmup MMs pad DMA waits ----
        qts_cur = []
        drain(
            roundrobin(
                warmup_items(16),
                chain(projKV_items(0, xts[0]), projQ_items(0, xts[0], qts_cur)),
            )
        )

        op_gen = None  # outproj of the previous block, used as filler
        for ib in range(NBLK):
            if ib + 2 < NBLK:
                load_xt(ib + 2)
            qts_next = []
            # Filler for this block's attention: K/V of THIS block first
            # (needed by its diagonal groups, so it gets absolute priority),
            # then outproj of the previous block round-robined with Q (and
            # possibly K/V) of the next block. K/V of block ib+1 is withheld
            # from block ib when it can instead feed block ib+1's own
            # attention (keeps the last block's PE fed).
            others = []
            supply = 0
            if ib + 1 < NBLK:
                nxt = [projQ_items(ib + 1, xts[ib + 1], qts_next)]
                supply += 20
                if ib + 1 < 2:
                    nxt.insert(0, projKV_items(ib + 1, xts[ib + 1]))
                    supply += 17
                others.append(chain(*nxt))
            if op_gen is not None:
                others.append(op_gen)
                supply += 16
            gen = roundrobin(*others) if others else iter(())
            if ib >= 2:
                gen = chain(projKV_items(ib, xts[ib]), gen)
                supply += 17

            def fill(n):
                for _ in range(n):
                    next(gen, None)

            # pace the filler so it lasts the whole block (a reserve is kept
            # for the normalize chain at block end)
            sites = 4 * (ib + 1) * (IB // P)
            rate = max(supply - 6, 0) / sites
            carry = [0.0]

            def fill_site():
                carry[0] += rate
                while carry[0] >= 1.0:
                    carry[0] -= 1.0
                    next(gen, None)

            # ============ attention for this query block ============
            njt = (ib + 1) * (IB // P)  # causal: key tiles 0..njt-1
            # last diag group(s) feed the denominator via direct PE matmuls
            # (skips the DVE accumulator on the block's critical tail)
            direct_from = 2 if ib == NBLK - 1 else 3
            ps_ctx = psum_cb.tile([P, REP, IB], F32, name=f"ctx{ib}", tag="cb")
            acc = work.tile([P, REP, IB], F32, name=f"acc{ib}", tag="acc", bufs=1)
            den = None

            # ctx matmuls lag the scores/exp by one key tile: by the time
            # ctx(jk-1) is at the head of the in-order PE queue, exp(jk-1)
            # finished long ago, so the exp latency never stalls the PE.
            def emit_ctx_half(pend, h):
                p_jk, p_ex, p_i0 = pend
                for rr in (0, 1):
                    r = 2 * h + rr
                    nc.tensor.matmul(
                        ps_ctx[:, r, p_i0:],
                        lhsT=v_sb[p_jk],
                        rhs=p_ex[:, r, p_i0:],
                        start=(p_jk == 0), stop=(p_jk == njt - 1),
                        skip_group_check=True,
                    )

            def emit_den_direct(pend):
                p_jk, p_ex, p_i0 = pend
                p_m = p_jk - (njt - 4)
                if p_m < direct_from:
                    return
                for r in range(REP):
                    nc.tensor.matmul(
                        den[:, p_i0:],
                        lhsT=sel_ones[r],
                        rhs=p_ex[:, r, p_i0:],
                        start=False, stop=(p_m == 3 and r == REP - 1),
                        skip_group_check=True,
                    )

            pend = None
            for jk in range(njt):
                m = jk - (njt - 4)  # >= 0 on the diagonal strip
                i0 = max(m, 0) * P  # live columns: i >= 128*m
                ex = work.tile(
                    [P, REP, IB], BF16, name=f"ex{ib}_{jk}", tag="ex", bufs=4
                )
                for h in (0, 1):
                    sb = psum_sb.tile(
                        [P, 2, IB], F32, name=f"sb{ib}_{jk}_{h}", tag="sb"
                    )
                    for rr in (0, 1):
                        r = 2 * h + rr
                        nc.tensor.matmul(
                            sb[:, rr, i0:],
                            lhsT=kT[:, jk * P : (jk + 1) * P],
                            rhs=qts_cur[r][:, i0:],
                            start=True, stop=True, skip_group_check=True,
                        )
                    nc.scalar.activation(
                        ex[:, 2 * h : 2 * h + 2, i0:],
                        sb[:, :, i0:],
                        mybir.ActivationFunctionType.Exp,
                        scale=SCALE,
                    )
                    if m >= 0:
                        for rr in (0, 1):
                            r = 2 * h + rr
                            # triangular strip: keep where (i - i0) - j >= 0
                            nc.gpsimd.affine_select(
                                out=ex[:, r, i0 : i0 + P],
                                in_=ex[:, r, i0 : i0 + P],
                                compare_op=mybir.AluOpType.is_ge,
                                fill=0.0,
                                base=0,
                                pattern=[[1, P]],
                                channel_multiplier=-1,
                            )
                    fill_site()
                    if pend is not None:
                        emit_ctx_half(pend, h)
                    fill_site()
                if pend is not None:
                    emit_den_direct(pend)
                if m < direct_from:
                    # DVE owns the accumulator (Pool measured 2x slower per
                    # add and the chain is serial per engine).
                    if jk == 0:
                        nc.vector.tensor_copy(acc, ex)
                    else:
                        nc.vector.tensor_add(
                            acc[:, :, i0:], acc[:, :, i0:], ex[:, :, i0:]
                        )
                if m == direct_from - 1:
                    # denominator from the accumulator; diag tail comes from
                    # direct matmuls on the remaining ex tiles (emit_den_direct)
                    acc_b = work.tile(
                        [P, REP, IB], BF16, name=f"accb{ib}", tag="accb", bufs=1
                    )
                    nc.vector.tensor_copy(acc_b, acc)
                    den = psum_pp.tile([P, IB], F32, name=f"den{ib}", tag="pp")
                    for r in range(REP):
                        nc.tensor.matmul(
                            den,
                            lhsT=sel_ones[r],
                            rhs=acc_b[:, r, :],
                            start=(r == 0), stop=False,
                            skip_group_check=True,
                        )
                pend = (jk, ex, i0)
            # flush the lagged ctx + diag denominator of the last key tile
            emit_ctx_half(pend, 0)
            emit_ctx_half(pend, 1)
            emit_den_direct(pend)

            # ============ normalize: recip + broadcast ============
            rec4 = work.tile([REP, IB], F32, name=f"rec4{ib}", tag="rec4", bufs=2)
            nc.vector.reciprocal_approx_fast(out=rec4, in_=den[0:REP, :])
            nc.vector.tensor_copy(rec4b[0:REP, :], rec4)
            fill(2)
            cns = []
            for h in (0, 1):
                rbp = psum_sb.tile([P, 2, IB], F32, name=f"rb{ib}_{h}", tag="sb")
                for rr in (0, 1):
                    nc.tensor.matmul(
                        rbp[:, rr, :], lhsT=sel4[2 * h + rr], rhs=rec4b,
                        start=True, stop=True, skip_group_check=True,
                    )
                rbs = work.tile(
                    [P, 2, IB], BF16, name=f"rbs{ib}_{h}", tag="rbs", bufs=2
                )
                nc.scalar.copy(rbs, rbp)
                fill(2)
                for rr in (0, 1):
                    r = 2 * h + rr
                    cn = work.tile(
                        [P, IB], BF16, name=f"cn{ib}_{r}", tag="cn", bufs=8
                    )
                    nc.vector.tensor_mul(cn, ps_ctx[:, r, :], rbs[:, rr, :])
                    cns.append(cn)
            drain(gen)
            op_gen = outproj_items(ib, cns)
            qts_cur = qts_next

        drain(op_gen)


_NC_CACHE = None


def kernel(x, Wq, Wk, Wv, Wo, bo):
    global _LAST_RESULT, _NC_CACHE
    x = np.asarray(x, dtype=np.float32)
    Wq = np.asarray(Wq, dtype=np.float32)
    Wk = np.asarray(Wk, dtype=np.float32)
    Wv = np.asarray(Wv, dtype=np.float32)
    Wo = np.asarray(Wo, dtype=np.float32)
    bo = np.asarray(bo, dtype=np.float32)

    if _NC_CACHE is None:
        _NC_CACHE = build_bass()
    nc = _NC_CACHE

    def chunked(a, pdim):
        # [pdim*nchunk, F] -> [pdim, nchunk, F] bf16, partition-major
        nchunk = a.shape[0] // pdim
        return np.ascontiguousarray(
            a.reshape(nchunk, pdim, a.shape[1]).transpose(1, 0, 2)
        ).astype(ml_dtypes.bfloat16)

    in_maps = []
    for core in range(8):
        b, g = core // G, core % G
        in_maps.append(
            {
                "xT": chunked(np.ascontiguousarray(x[b].T), P),
                "wq": chunked(Wq[:, g * E : (g + 1) * E], P),
                "wk": chunked(Wk[:, g * HD : (g + 1) * HD], P),
                "wv": chunked(Wv[:, g * HD : (g + 1) * HD], P),
                "wo": chunked(Wo[g * E : (g + 1) * E, :], P),
            }
        )
    res = run_bass_kernel_spmd(
        nc,
        in_maps,
        list(range(8)),
        trace=bool(os.environ.get("BASS_TRACE")),
    )
    _LAST_RESULT = res
    partials = np.stack(
        [np.asarray(r["out"]).astype(np.float32) for r in res.results]
    )  # [8, N, D]
    full = partials.reshape(B, G, N, D).sum(axis=1) + bo[None, None, :]
    return full.astype(np.float32)
